# revision 6
# baseline (speedup 1.0000x reference)
"""Trainium2 Bass kernel for nn_Encoder_inter: coif1 wavelet disentangle along
the node axis (expressed as a dense 512x512 matrix, precomputed on host) followed
by a 2-layer MLP (64->256->256) with ReLU, pointwise over (B, N, T).

Sharding: data-parallel over batch B=32 across 8 NeuronCores (4 batches each);
the small Linear weights and the wavelet matrix are replicated.
"""
import os
import sys

for _p in ("/opt/trn_rl_repo", "/root/.axon_site/_ro/trn_rl_repo"):
    if os.path.isdir(_p) and _p not in sys.path:
        sys.path.insert(0, _p)

from contextlib import ExitStack

import numpy as np

import concourse.bass as bass
import concourse.tile as tile
from concourse import bacc, mybir
from concourse.bass_utils import run_bass_kernel_spmd

F32 = mybir.dt.float32
F32R = mybir.dt.float32r

B, N, T, D, H, G = 32, 512, 24, 64, 256, 256
NCORES = 8
BPC = B // NCORES          # batches per core
TD = T * D                 # 1536
NCHUNK = N // 128          # 4
MCHUNK = N // 128          # 4
THALF = T // 2             # 12

# ---------------------------------------------------------------------------
# Host-side wavelet matrix: the whole dwt -> (2*cD) -> idwt chain along the
# node axis is linear, so it is exactly y = K @ x with K (N, N). We build
# K^T = op(eye(N)) in float64 with a numpy port of the reference transform.
# ---------------------------------------------------------------------------
_L = 6
_DEC_LO = np.array(
    [-0.01565572813546454, -0.0727326195128539, 0.38486484686420286,
     0.8525720202122554, 0.3378976624578092, -0.0727326195128539],
    dtype=np.float64,
)
_DEC_HI = np.array(
    [0.0727326195128539, 0.3378976624578092, -0.8525720202122554,
     0.38486484686420286, 0.0727326195128539, -0.01565572813546454],
    dtype=np.float64,
)
_REC_LO = _DEC_LO[::-1].copy()
_REC_HI = _DEC_HI[::-1].copy()


def _dwt_last(x):
    n = x.shape[-1]
    ext = np.concatenate(
        [x[..., : _L - 1][..., ::-1], x, x[..., -(_L - 1):][..., ::-1]], axis=-1
    )
    out = (n + _L - 2) // 2
    cA = sum(_DEC_LO[j] * ext[..., _L - j: _L - j + 2 * out: 2] for j in range(_L))
    cD = sum(_DEC_HI[j] * ext[..., _L - j: _L - j + 2 * out: 2] for j in range(_L))
    return cA, cD


def _idwt_last(cA, cD, n):
    out = cA.shape[-1]
    up_shape = cA.shape[:-1] + (2 * out - 1,)
    upA = np.zeros(up_shape, cA.dtype)
    upA[..., ::2] = cA
    upD = np.zeros(up_shape, cD.dtype)
    upD[..., ::2] = cD
    pad = [(0, 0)] * (cA.ndim - 1) + [(_L - 1, _L - 1)]
    uA = np.pad(upA, pad)
    uD = np.pad(upD, pad)
    return sum(
        _REC_LO[j] * uA[..., 2 * _L - 3 - j: 2 * _L - 3 - j + n]
        + _REC_HI[j] * uD[..., 2 * _L - 3 - j: 2 * _L - 3 - j + n]
        for j in range(_L)
    )


def _wavelet_kt() -> np.ndarray:
    """K^T (m_in, n_out) so that (op(x))[n] = sum_m x[m] * KT[m, n]."""
    eye = np.eye(N, dtype=np.float64)
    cA, cD = _dwt_last(eye)
    kt = _idwt_last(cA, 2.0 * cD, N)
    return kt.astype(np.float32)


# ---------------------------------------------------------------------------
# Device kernel (SPMD, identical program on all 8 cores)
# ---------------------------------------------------------------------------
_NC_CACHE = None


def _build_nc():
    nc = bacc.Bacc("TRN2", target_bir_lowering=False, debug=False, num_devices=NCORES)
    x_d = nc.dram_tensor("x", [BPC, MCHUNK, 128, TD], F32, kind="ExternalInput").ap()
    kt_d = nc.dram_tensor("KT", [MCHUNK, 128, N], F32, kind="ExternalInput").ap()
    w1_d = nc.dram_tensor("W1T", [D, H], F32, kind="ExternalInput").ap()
    w2_d = nc.dram_tensor("W2T", [2, 128, G], F32, kind="ExternalInput").ap()
    b1_d = nc.dram_tensor("b1", [2, 128, 1], F32, kind="ExternalInput").ap()
    b2_d = nc.dram_tensor("b2", [1, G], F32, kind="ExternalInput").ap()
    ones_d = nc.dram_tensor("ones", [1, 128], F32, kind="ExternalInput").ap()
    out_d = nc.dram_tensor("out", [BPC, N, T, G], F32, kind="ExternalOutput").ap()

    relu = mybir.ActivationFunctionType.Relu

    with tile.TileContext(nc) as tc, ExitStack() as ctx:
        consts = ctx.enter_context(tc.tile_pool(name="consts", bufs=1))
        xpool = ctx.enter_context(tc.tile_pool(name="xp", bufs=2))
        ypool = ctx.enter_context(tc.tile_pool(name="yp", bufs=3))
        hpool = ctx.enter_context(tc.tile_pool(name="hp", bufs=2))
        spool = ctx.enter_context(tc.tile_pool(name="sp", bufs=2))
        py = ctx.enter_context(tc.tile_pool(name="py", bufs=2, space="PSUM"))
        ph = ctx.enter_context(tc.tile_pool(name="ph", bufs=2, space="PSUM"))
        po = ctx.enter_context(tc.tile_pool(name="po", bufs=4, space="PSUM"))

        # --- replicated constants ---
        kt_sb = []
        for mc in range(MCHUNK):
            t_ = consts.tile([128, N], F32R, tag=f"kt{mc}", name=f"kt{mc}")
            nc.sync.dma_start(out=t_[:], in_=kt_d[mc].bitcast(F32R))
            kt_sb.append(t_)
        w1_sb = consts.tile([D, H], F32R, tag="w1", name="w1")
        nc.sync.dma_start(out=w1_sb[:], in_=w1_d[:].bitcast(F32R))
        w2_sb = []
        for hc in range(2):
            t_ = consts.tile([128, G], F32R, tag=f"w2{hc}", name=f"w2{hc}")
            nc.sync.dma_start(out=t_[:], in_=w2_d[hc].bitcast(F32R))
            w2_sb.append(t_)
        b1_sb = []
        for hc in range(2):
            t_ = consts.tile([128, 1], F32, tag=f"b1{hc}", name=f"b1c{hc}")
            nc.sync.dma_start(out=t_[:], in_=b1_d[hc])
            b1_sb.append(t_)
        b2_sb = consts.tile([1, G], F32R, tag="b2", name="b2c")
        nc.sync.dma_start(out=b2_sb[:], in_=b2_d[:].bitcast(F32R))
        ones_sb = consts.tile([1, 128], F32R, tag="ones", name="ones")
        nc.sync.dma_start(out=ones_sb[:], in_=ones_d[:].bitcast(F32R))

        for b in range(BPC):
            x_sb = []
            for mc in range(MCHUNK):
                t_ = xpool.tile([128, TD], F32R, tag=f"x{mc}", name=f"xt{mc}")
                nc.sync.dma_start(out=t_[:], in_=x_d[b, mc].bitcast(F32R))
                x_sb.append(t_)
            for half in range(2):
                stg = [
                    spool.tile([128, THALF * G], F32, tag=f"stg{nck}", name=f"stg{nck}")
                    for nck in range(NCHUNK)
                ]
                for tl in range(THALF):
                    t = half * THALF + tl
                    # step 1: y^T (d, n) = sum_m x[m, d] * KT[m, n]
                    yps = py.tile([D, N], F32, name="yps")
                    for mc in range(MCHUNK):
                        nc.tensor.matmul(
                            yps[:],
                            lhsT=x_sb[mc][:, t * D:(t + 1) * D],
                            rhs=kt_sb[mc][:],
                            start=(mc == 0),
                            stop=(mc == MCHUNK - 1),
                        )
                    y_sb = ypool.tile([D, N], F32R, tag="yt", name="y_sb")
                    nc.scalar.copy(y_sb[:], yps[:])
                    # step 2: h1^T (h, n) = relu(W1 @ y^T + b1)
                    h1 = []
                    for hc in range(2):
                        hps = ph.tile([128, N], F32, name="hps")
                        nc.tensor.matmul(
                            hps[:],
                            lhsT=w1_sb[:, hc * 128:(hc + 1) * 128],
                            rhs=y_sb[:],
                            start=True,
                            stop=True,
                        )
                        h_sb = hpool.tile([128, N], F32R, tag=f"h1_{hc}", name=f"h1_{hc}")
                        nc.scalar.activation(
                            h_sb[:], hps[:], relu, bias=b1_sb[hc][:], scale=1.0
                        )
                        h1.append(h_sb)
                    # step 3: out (n, g) = relu(h1 @ W2^T + b2)
                    for nck in range(NCHUNK):
                        ops = po.tile([128, G], F32, name="ops")
                        nc.tensor.matmul(
                            ops[:],
                            lhsT=ones_sb[:],
                            rhs=b2_sb[:],
                            start=True,
                            stop=False,
                            skip_group_check=True,
                        )
                        for hc in range(2):
                            nc.tensor.matmul(
                                ops[:],
                                lhsT=h1[hc][:, nck * 128:(nck + 1) * 128],
                                rhs=w2_sb[hc][:],
                                start=False,
                                stop=(hc == 1),
                                skip_group_check=True,
                            )
                        nc.vector.tensor_scalar_max(
                            stg[nck][:, tl * G:(tl + 1) * G], ops[:], 0.0
                        )
                for nck in range(NCHUNK):
                    nc.sync.dma_start(
                        out=out_d[
                            b,
                            nck * 128:(nck + 1) * 128,
                            half * THALF:(half + 1) * THALF,
                            :,
                        ],
                        in_=stg[nck][:].rearrange("p (t g) -> p t g", t=THALF),
                    )
    nc.compile()
    return nc


def _get_nc():
    global _NC_CACHE
    if _NC_CACHE is None:
        _NC_CACHE = _build_nc()
    return _NC_CACHE


def _make_in_maps(x, W1, b1, W2, b2):
    x = np.ascontiguousarray(np.asarray(x, dtype=np.float32))
    W1 = np.asarray(W1, dtype=np.float32)
    b1 = np.asarray(b1, dtype=np.float32)
    W2 = np.asarray(W2, dtype=np.float32)
    b2 = np.asarray(b2, dtype=np.float32)

    kt = _wavelet_kt().reshape(MCHUNK, 128, N)
    w1t = np.ascontiguousarray(W1.T)                      # (D, H)
    w2t = np.ascontiguousarray(W2.T).reshape(2, 128, G)    # (h, g) chunks
    b1r = np.ascontiguousarray(b1.reshape(2, 128, 1))
    b2r = np.ascontiguousarray(b2.reshape(1, G))
    ones = np.ones((1, 128), dtype=np.float32)

    in_maps = []
    for c in range(NCORES):
        xc = x[c * BPC:(c + 1) * BPC].reshape(BPC, N, TD)
        xc = np.ascontiguousarray(xc.reshape(BPC, MCHUNK, 128, TD))
        in_maps.append(
            {"x": xc, "KT": kt, "W1T": w1t, "W2T": w2t, "b1": b1r, "b2": b2r,
             "ones": ones}
        )
    return in_maps


def kernel(x, W1, b1, W2, b2):
    nc = _get_nc()
    in_maps = _make_in_maps(x, W1, b1, W2, b2)
    res = run_bass_kernel_spmd(nc, in_maps, list(range(NCORES)))
    out = np.concatenate([res.results[c]["out"] for c in range(NCORES)], axis=0)
    return out


# revision 8
# speedup vs baseline: 1.1602x; 1.1602x over previous
"""Trainium2 Bass kernel for nn_Encoder_inter: coif1 wavelet disentangle along
the node axis (expressed as a dense 512x512 matrix, precomputed on host) followed
by a 2-layer MLP (64->256->256) with ReLU, pointwise over (B, N, T).

Sharding: data-parallel over batch B=32 across 8 NeuronCores (4 batches each);
the small Linear weights and the wavelet matrix are replicated.
"""
import os
import sys

for _p in ("/opt/trn_rl_repo", "/root/.axon_site/_ro/trn_rl_repo"):
    if os.path.isdir(_p) and _p not in sys.path:
        sys.path.insert(0, _p)

from contextlib import ExitStack

import numpy as np

import concourse.bass as bass
import concourse.tile as tile
from concourse import bacc, mybir
from concourse.bass_utils import run_bass_kernel_spmd

F32 = mybir.dt.float32
F32R = mybir.dt.float32r
BF16 = mybir.dt.bfloat16

# compute dtype for tensor-engine operands: "bf16" or "f32r"
COMPUTE = os.environ.get("KERNEL_COMPUTE_DTYPE", "bf16")
MM_DT = BF16 if COMPUTE == "bf16" else F32R

B, N, T, D, H, G = 32, 512, 24, 64, 256, 256
NCORES = 8
BPC = B // NCORES          # batches per core
TD = T * D                 # 1536
NCHUNK = N // 128          # 4
MCHUNK = N // 128          # 4
THALF = T // 2             # 12

# ---------------------------------------------------------------------------
# Host-side wavelet matrix: the whole dwt -> (2*cD) -> idwt chain along the
# node axis is linear, so it is exactly y = K @ x with K (N, N). We build
# K^T = op(eye(N)) in float64 with a numpy port of the reference transform.
# ---------------------------------------------------------------------------
_L = 6
_DEC_LO = np.array(
    [-0.01565572813546454, -0.0727326195128539, 0.38486484686420286,
     0.8525720202122554, 0.3378976624578092, -0.0727326195128539],
    dtype=np.float64,
)
_DEC_HI = np.array(
    [0.0727326195128539, 0.3378976624578092, -0.8525720202122554,
     0.38486484686420286, 0.0727326195128539, -0.01565572813546454],
    dtype=np.float64,
)
_REC_LO = _DEC_LO[::-1].copy()
_REC_HI = _DEC_HI[::-1].copy()


def _dwt_last(x):
    n = x.shape[-1]
    ext = np.concatenate(
        [x[..., : _L - 1][..., ::-1], x, x[..., -(_L - 1):][..., ::-1]], axis=-1
    )
    out = (n + _L - 2) // 2
    cA = sum(_DEC_LO[j] * ext[..., _L - j: _L - j + 2 * out: 2] for j in range(_L))
    cD = sum(_DEC_HI[j] * ext[..., _L - j: _L - j + 2 * out: 2] for j in range(_L))
    return cA, cD


def _idwt_last(cA, cD, n):
    out = cA.shape[-1]
    up_shape = cA.shape[:-1] + (2 * out - 1,)
    upA = np.zeros(up_shape, cA.dtype)
    upA[..., ::2] = cA
    upD = np.zeros(up_shape, cD.dtype)
    upD[..., ::2] = cD
    pad = [(0, 0)] * (cA.ndim - 1) + [(_L - 1, _L - 1)]
    uA = np.pad(upA, pad)
    uD = np.pad(upD, pad)
    return sum(
        _REC_LO[j] * uA[..., 2 * _L - 3 - j: 2 * _L - 3 - j + n]
        + _REC_HI[j] * uD[..., 2 * _L - 3 - j: 2 * _L - 3 - j + n]
        for j in range(_L)
    )


def _wavelet_kt() -> np.ndarray:
    """K^T (m_in, n_out) so that (op(x))[n] = sum_m x[m] * KT[m, n]."""
    eye = np.eye(N, dtype=np.float64)
    cA, cD = _dwt_last(eye)
    kt = _idwt_last(cA, 2.0 * cD, N)
    return kt.astype(np.float32)


# ---------------------------------------------------------------------------
# Device kernel (SPMD, identical program on all 8 cores)
# ---------------------------------------------------------------------------
_NC_CACHE = None


def _build_nc():
    nc = bacc.Bacc("TRN2", target_bir_lowering=False, debug=False, num_devices=NCORES)
    x_d = nc.dram_tensor("x", [BPC, MCHUNK, 128, TD], MM_DT, kind="ExternalInput").ap()
    kt_d = nc.dram_tensor("KT", [MCHUNK, 128, N], MM_DT, kind="ExternalInput").ap()
    w1_d = nc.dram_tensor("W1T", [D, H], MM_DT, kind="ExternalInput").ap()
    w2_d = nc.dram_tensor("W2T", [2, 128, G], MM_DT, kind="ExternalInput").ap()
    b1_d = nc.dram_tensor("b1", [2, 128, 1], F32, kind="ExternalInput").ap()
    b2_d = nc.dram_tensor("b2", [1, G], MM_DT, kind="ExternalInput").ap()
    ones_d = nc.dram_tensor("ones", [1, 128], MM_DT, kind="ExternalInput").ap()
    out_d = nc.dram_tensor("out", [BPC, N, T, G], F32, kind="ExternalOutput").ap()

    relu = mybir.ActivationFunctionType.Relu

    with tile.TileContext(nc) as tc, ExitStack() as ctx:
        consts = ctx.enter_context(tc.tile_pool(name="consts", bufs=1))
        xpool = ctx.enter_context(tc.tile_pool(name="xp", bufs=2))
        ypool = ctx.enter_context(tc.tile_pool(name="yp", bufs=3))
        hpool = ctx.enter_context(tc.tile_pool(name="hp", bufs=2))
        spool = ctx.enter_context(tc.tile_pool(name="sp", bufs=2))
        py = ctx.enter_context(tc.tile_pool(name="py", bufs=2, space="PSUM"))
        ph = ctx.enter_context(tc.tile_pool(name="ph", bufs=2, space="PSUM"))
        po = ctx.enter_context(tc.tile_pool(name="po", bufs=4, space="PSUM"))

        # --- replicated constants ---
        kt_sb = []
        for mc in range(MCHUNK):
            t_ = consts.tile([128, N], MM_DT, tag=f"kt{mc}", name=f"kt{mc}")
            nc.sync.dma_start(out=t_[:], in_=kt_d[mc])
            kt_sb.append(t_)
        w1_sb = consts.tile([D, H], MM_DT, tag="w1", name="w1")
        nc.sync.dma_start(out=w1_sb[:], in_=w1_d[:])
        w2_sb = []
        for hc in range(2):
            t_ = consts.tile([128, G], MM_DT, tag=f"w2{hc}", name=f"w2{hc}")
            nc.sync.dma_start(out=t_[:], in_=w2_d[hc])
            w2_sb.append(t_)
        b1_sb = []
        for hc in range(2):
            t_ = consts.tile([128, 1], F32, tag=f"b1{hc}", name=f"b1c{hc}")
            nc.sync.dma_start(out=t_[:], in_=b1_d[hc])
            b1_sb.append(t_)
        b2_sb = consts.tile([1, G], MM_DT, tag="b2", name="b2c")
        nc.sync.dma_start(out=b2_sb[:], in_=b2_d[:])
        ones_sb = consts.tile([1, 128], MM_DT, tag="ones", name="ones")
        nc.sync.dma_start(out=ones_sb[:], in_=ones_d[:])

        for b in range(BPC):
            x_sb = []
            for mc in range(MCHUNK):
                t_ = xpool.tile([128, TD], MM_DT, tag=f"x{mc}", name=f"xt{mc}")
                nc.sync.dma_start(out=t_[:], in_=x_d[b, mc])
                x_sb.append(t_)
            for half in range(2):
                stg = [
                    spool.tile([128, THALF * G], F32, tag=f"stg{nck}", name=f"stg{nck}")
                    for nck in range(NCHUNK)
                ]
                for tl in range(THALF):
                    t = half * THALF + tl
                    # step 1: y^T (d, n) = sum_m x[m, d] * KT[m, n]
                    yps = py.tile([D, N], F32, name="yps")
                    for mc in range(MCHUNK):
                        nc.tensor.matmul(
                            yps[:],
                            lhsT=x_sb[mc][:, t * D:(t + 1) * D],
                            rhs=kt_sb[mc][:],
                            start=(mc == 0),
                            stop=(mc == MCHUNK - 1),
                        )
                    y_sb = ypool.tile([D, N], MM_DT, tag="yt", name="y_sb")
                    nc.scalar.copy(y_sb[:], yps[:])
                    # step 2: h1^T (h, n) = relu(W1 @ y^T + b1)
                    h1 = []
                    for hc in range(2):
                        hps = ph.tile([128, N], F32, name="hps")
                        nc.tensor.matmul(
                            hps[:],
                            lhsT=w1_sb[:, hc * 128:(hc + 1) * 128],
                            rhs=y_sb[:],
                            start=True,
                            stop=True,
                        )
                        h_sb = hpool.tile([128, N], MM_DT, tag=f"h1_{hc}", name=f"h1_{hc}")
                        nc.scalar.activation(
                            h_sb[:], hps[:], relu, bias=b1_sb[hc][:], scale=1.0
                        )
                        h1.append(h_sb)
                    # step 3: out (n, g) = relu(h1 @ W2^T + b2)
                    for nck in range(NCHUNK):
                        ops = po.tile([128, G], F32, name="ops")
                        nc.tensor.matmul(
                            ops[:],
                            lhsT=ones_sb[:],
                            rhs=b2_sb[:],
                            start=True,
                            stop=False,
                            skip_group_check=True,
                        )
                        for hc in range(2):
                            nc.tensor.matmul(
                                ops[:],
                                lhsT=h1[hc][:, nck * 128:(nck + 1) * 128],
                                rhs=w2_sb[hc][:],
                                start=False,
                                stop=(hc == 1),
                                skip_group_check=True,
                            )
                        nc.vector.tensor_scalar_max(
                            stg[nck][:, tl * G:(tl + 1) * G], ops[:], 0.0
                        )
                for nck in range(NCHUNK):
                    nc.sync.dma_start(
                        out=out_d[
                            b,
                            nck * 128:(nck + 1) * 128,
                            half * THALF:(half + 1) * THALF,
                            :,
                        ],
                        in_=stg[nck][:].rearrange("p (t g) -> p t g", t=THALF),
                    )
    nc.compile()
    return nc


def _get_nc():
    global _NC_CACHE
    if _NC_CACHE is None:
        _NC_CACHE = _build_nc()
    return _NC_CACHE


def _make_in_maps(x, W1, b1, W2, b2):
    if COMPUTE == "bf16":
        import ml_dtypes
        mmnp = ml_dtypes.bfloat16
    else:
        mmnp = np.float32
    x = np.ascontiguousarray(np.asarray(x, dtype=np.float32))
    W1 = np.asarray(W1, dtype=np.float32)
    b1 = np.asarray(b1, dtype=np.float32)
    W2 = np.asarray(W2, dtype=np.float32)
    b2 = np.asarray(b2, dtype=np.float32)

    kt = _wavelet_kt().reshape(MCHUNK, 128, N).astype(mmnp)
    w1t = np.ascontiguousarray(W1.T).astype(mmnp)              # (D, H)
    w2t = np.ascontiguousarray(W2.T).reshape(2, 128, G).astype(mmnp)
    b1r = np.ascontiguousarray(b1.reshape(2, 128, 1))
    b2r = np.ascontiguousarray(b2.reshape(1, G)).astype(mmnp)
    ones = np.ones((1, 128), dtype=mmnp)

    in_maps = []
    for c in range(NCORES):
        xc = x[c * BPC:(c + 1) * BPC].reshape(BPC, N, TD)
        xc = np.ascontiguousarray(xc.reshape(BPC, MCHUNK, 128, TD).astype(mmnp))
        in_maps.append(
            {"x": xc, "KT": kt, "W1T": w1t, "W2T": w2t, "b1": b1r, "b2": b2r,
             "ones": ones}
        )
    return in_maps


def kernel(x, W1, b1, W2, b2):
    nc = _get_nc()
    in_maps = _make_in_maps(x, W1, b1, W2, b2)
    res = run_bass_kernel_spmd(nc, in_maps, list(range(NCORES)))
    out = np.concatenate([res.results[c]["out"] for c in range(NCORES)], axis=0)
    return out
